# revision 13
# baseline (speedup 1.0000x reference)
"""Causal self-attention (B=8, T=2048, C=128, H=4, D=32) on 8 trn2 NeuronCores.

Sharding: data-parallel over batch - core b handles batch element b.

Per-core algorithm (all PE matmuls bf16; transposes fp32):
  xT = transpose(x) in bf16            # [C, T]
  qT, kT = (x @ Wq|k + b)^T bf16       # [C, {q,k}, T]
  v   = x @ Wv + bv                    # packed to vaug [128, NT, 4, 33]
                                       # (32 v dims + ones col = denominator)
  flat pipeline over (tq-block j, head pair, tk-tile a <= 4j+3):
      S^T[tk,tq] = kT_h.T @ qT_h       # K=32 row-packed, PSUM [128,1024]
      E = exp(S/sqrt(32))              # split: ACT exact exp | DVE 1-op
                                       # Schraudolph (int16 bitcast bf16)
      (diag: gpsimd affine_select zeroes the causal triangle of E)
      PV transposed, per (head, tq-tile m): psum_y[tq, h, 0:33] +=
          E-chunk[tk,tq-tile].T @ vaug[tk, h, 0:33]
  per j: denominators are psum_y[:, :, :, 32] (per-partition!) ->
  DVE reciprocal + broadcast-mult -> ynorm [tq, (h d)] fp32;
  PE transpose per tq-tile -> yT [(h d), tq] bf16; single proj matmul
  vs full w_proj bf16; bias add; DMA out. Proj work runs one block
  behind the attention pipeline as deferred background work.
"""

import sys

sys.path.insert(0, "/opt/trn_rl_repo")

import numpy as np

B, T, C = 8, 2048, 128
H, D = 4, 32
N_CORES = 8
TQ = 512          # tq block
NT = T // 128     # 16 tk tiles
NJ = T // TQ      # 4 tq blocks
SCALE = 1.0 / np.sqrt(D)
# Schraudolph exp: bf16(int16(s*A + B)); fold 1/sqrt(D) into A
A_SCH = float(SCALE * 128.0 / np.log(2.0))
B_SCH = float(127.0 * 128.0 - 7.42)
DVE_EXP_NUM = 5       # route 5/20 of exp items to DVE schraudolph
DVE_EXP_DEN = 20

_cache = {}


def _build():
    import concourse.bass as bass
    import concourse.mybir as mybir
    import concourse.tile as tile
    from concourse import bacc
    from concourse.masks import make_identity

    dt = mybir.dt
    AF = mybir.ActivationFunctionType
    nc = bacc.Bacc()

    x = nc.dram_tensor("x", [T, C], dt.float32, kind="ExternalInput")
    w_qkv = nc.dram_tensor("w_qkv", [C, 3 * C], dt.float32, kind="ExternalInput")
    b_qkv = nc.dram_tensor("b_qkv", [3 * C], dt.float32, kind="ExternalInput")
    w_proj = nc.dram_tensor("w_proj", [C, C], dt.float32, kind="ExternalInput")
    b_proj = nc.dram_tensor("b_proj", [C], dt.float32, kind="ExternalInput")
    y = nc.dram_tensor("y", [T, C], dt.float32, kind="ExternalOutput")

    with tile.TileContext(nc) as tc:
        with (
            nc.allow_low_precision(reason="bf16 matmuls + partial schraudolph exp; validated vs ref"),
            tc.tile_pool(name="const", bufs=1) as const,
            tc.tile_pool(name="big", bufs=1) as big,
            tc.tile_pool(name="sb", bufs=4) as sb,
            tc.tile_pool(name="esb", bufs=8) as esb,
            tc.tile_pool(name="ysb", bufs=3) as ysb,
            tc.tile_pool(name="ps_misc", bufs=2, space="PSUM") as ps_misc,
            tc.tile_pool(name="ps_s", bufs=2, space="PSUM") as ps_s,
            tc.tile_pool(name="ps_y", bufs=2, space="PSUM") as ps_y,
        ):
            # ---------------- critical-path constants ----------------
            ident = const.tile([128, 128], dt.float32)
            make_identity(nc, ident)

            # dummy exp so the ACT table set loads early
            dumm = const.tile([1, 1], dt.float32)
            nc.scalar.activation(dumm, ident[0:1, 0:1], AF.Exp)

            # persistent activations
            xT = big.tile([128, T], dt.bfloat16)        # [c, t]
            qkT = big.tile([128, 2, T], dt.bfloat16)    # [c, {q,k}, t]
            # vaug per tk-tile a, head h: [v_h (32) | 1.0] -> 33 wide
            vaug = big.tile([128, NT, 4, 33], dt.bfloat16)

            x_ts = {}

            def emit_xdma(a):
                x_t = sb.tile([128, 128], dt.float32, tag="xin",
                              name=f"x_t_{a}")
                nc.sync.dma_start(out=x_t, in_=x[128 * a:128 * (a + 1), :])
                x_ts[a] = x_t

            def emit_xtr(a):
                p_tr = ps_misc.tile([128, 128], dt.float32, tag="misc")
                nc.tensor.transpose(p_tr, x_ts.pop(a), ident)
                nc.vector.tensor_copy(xT[:, 128 * a:128 * (a + 1)], p_tr)

            def emit_x1(a):
                emit_xdma(a)
                emit_xtr(a)

            def emit_qk1(g, ch):
                p_qk = ps_misc.tile([128, TQ], dt.float32, tag="misc")
                nc.tensor.matmul(
                    p_qk,
                    w_r[:, 128 * ch:128 * (ch + 1)],
                    xT[:, TQ * g:TQ * (g + 1)],
                    start=True, stop=True,
                )
                nc.vector.tensor_scalar_add(
                    qkT[:, ch, TQ * g:TQ * (g + 1)], p_qk, bqk[:, ch:ch + 1]
                )

            def emit_v1(a):
                p_v = ps_misc.tile([128, 128], dt.float32, tag="misc")
                nc.tensor.matmul(
                    p_v,
                    xT[:, 128 * a:128 * (a + 1)],
                    w_v[:, :],
                    start=True, stop=True,
                )
                nc.vector.tensor_add(
                    vaug[:, a, :, 0:32],
                    p_v.rearrange("p (h d) -> p h d", h=4),
                    bvb.rearrange("p (h d) -> p h d", h=4),
                )

            def emit_xqk(g):
                for a in range(4 * g, 4 * g + 4):
                    emit_x1(a)
                emit_qk1(g, 0)
                emit_qk1(g, 1)

            def emit_v(g):
                for a in range(4 * g, 4 * g + 4):
                    emit_v1(a)

            # group-0 x tiles first in the DMA queue; weights right after
            for _a in range(4):
                emit_x1(_a)

            # biases: b_q/b_k as [128,1] per-partition columns
            bqk = const.tile([128, 2], dt.float32)
            nc.sync.dma_start(
                out=bqk, in_=b_qkv[0:256].rearrange("(j p) -> p j", p=128)
            )
            # weights in bf16
            w_sb = const.tile([128, 3 * C], dt.float32)
            nc.sync.dma_start(out=w_sb, in_=w_qkv[:, :])
            w_r = const.tile([128, 2 * C], dt.bfloat16)
            nc.vector.tensor_copy(w_r, w_sb[:, 0:256])
            w_v = const.tile([128, C], dt.bfloat16)
            nc.vector.tensor_copy(w_v, w_sb[:, 256:384])

            emit_qk1(0, 0)
            emit_qk1(0, 1)

            # ---------------- remaining constants ----------------
            wp_sb = const.tile([128, C], dt.float32)
            nc.sync.dma_start(out=wp_sb, in_=w_proj[:, :])
            wp_bf = const.tile([128, C], dt.bfloat16)
            nc.vector.tensor_copy(wp_bf, wp_sb)

            # broadcast tiles for free-dim biases (b_v, b_proj)
            brow = const.tile([1, 256], dt.float32)
            nc.sync.dma_start(out=brow[:, 0:128], in_=b_qkv[256:384][None, :])
            nc.sync.dma_start(out=brow[:, 128:256], in_=b_proj[:][None, :])
            brow_r = const.tile([1, 256], dt.float32r)
            nc.vector.tensor_copy(brow_r, brow)
            ones1_f = const.tile([1, 128], dt.float32)
            nc.vector.memset(ones1_f, 1.0)
            ones1 = const.tile([1, 128], dt.float32r)
            nc.vector.tensor_copy(ones1, ones1_f)
            p_b = ps_misc.tile([128, 256], dt.float32, tag="misc")
            nc.tensor.matmul(p_b, ones1, brow_r, start=True, stop=True)
            bvb = const.tile([128, 128], dt.float32)    # b_v broadcast
            bpb = const.tile([128, 128], dt.float32)    # b_proj broadcast
            nc.vector.tensor_copy(bvb, p_b[:, 0:128])
            nc.vector.tensor_copy(bpb, p_b[:, 128:256])

            nc.gpsimd.memset(vaug, 0.0)
            nc.gpsimd.memset(vaug[:, :, :, 32:33], 1.0)

            emit_v(0)
            emit_xqk(1)
            emit_v(1)

            # ---------------- attention pipeline ----------------
            pys = {}

            def emit_pvt(pend):
                # PV transposed: per (head, tq-tile m): stationary E chunk
                # [tk, tq-128], moving vaug [tk, 33] -> psum [tq, 33].
                # The py tile owns a full 2KB zero region: exactly one
                # start (first matmul) and one stop (last matmul) per tile;
                # in between, untouched bytes stay pending-zero so each
                # (m, ih) slice's first accumulate lands on zero.
                jp, pairp, e_p, a_p, r_p = pend
                py = pys[(jp, pairp)]
                first = (a_p == 0)
                last = (a_p == 4 * jp + 3)
                for ih in range(2):
                    h = 2 * pairp + ih
                    for m in range(max(r_p, 0), 4):
                        nc.tensor.matmul(
                            py[:, m, ih, 0:33],
                            e_p.rearrange("p (i f) -> p i f", i=2)[
                                :, ih, 128 * m:128 * (m + 1)],
                            vaug[:, a_p, h, :],
                            start=(first and ih == 0 and m == max(r_p, 0)),
                            stop=(last and ih == 1 and m == 3),
                            skip_group_check=True,
                        )

            def emit_norm_j(j):
                # denominators sit at free idx 32, per partition (= tq)
                ynb = ysb.tile([128, 4, 4, 32], dt.float32, tag="ynb",
                               name=f"ynb_{j}", bufs=2)
                for pair in range(2):
                    py = pys[(j, pair)]
                    r8 = ysb.tile([128, 4, 2, 1], dt.float32, tag="r8",
                                  name=f"r8_{j}_{pair}")
                    nc.vector.reciprocal(r8, py[:, :, :, 32:33])
                    rbc = bass.AP(
                        tensor=r8.tensor, offset=r8.offset,
                        ap=[r8.ap[0], [2, 4], [1, 2], [0, 32]],
                    )
                    nc.vector.tensor_tensor(
                        out=ynb[:, :, 2 * pair:2 * pair + 2, :],
                        in0=py[:, :, :, 0:32], in1=rbc,
                        op=mybir.AluOpType.mult,
                    )
                return ynb

            def emit_proj_m(j, m):
                ynb = ynbs[j]
                p_yt = ps_misc.tile([128, 128], dt.float32, tag="misc")
                nc.tensor.transpose(
                    p_yt, ynb[:, m, :, :].rearrange("p h d -> p (h d)"), ident
                )
                yt_sb = ysb.tile([128, 128], dt.bfloat16, tag="yt")
                nc.vector.tensor_copy(yt_sb, p_yt)
                p_o = ps_misc.tile([128, 128], dt.float32, tag="misc")
                nc.tensor.matmul(p_o, yt_sb, wp_bf, start=True, stop=True)
                o_t = sb.tile([128, 128], dt.float32, tag="out")
                nc.vector.tensor_add(o_t, p_o, bpb)
                t0 = TQ * j + 128 * m
                nc.sync.dma_start(out=y[t0:t0 + 128, :], in_=o_t)

            items = [
                (j, pair, a)
                for j in range(NJ)
                for pair in range(2)
                for a in range(4 * j + 4)
            ]
            pends = []
            ynbs = {}

            from collections import deque
            bg = deque()

            def flush(pend):
                emit_pvt(pend)
                jp, pairp, _, a_p, _ = pend
                if a_p == 4 * jp + 3 and pairp == 1:
                    ynbs[jp] = emit_norm_j(jp)
                    if jp + 2 < NJ:
                        g = jp + 2
                        for aa in range(4 * g, 4 * g + 4):
                            bg.append(lambda aa=aa: emit_xdma(aa))
                        for aa in range(4 * g, 4 * g + 4):
                            bg.append(lambda aa=aa: emit_xtr(aa))
                        bg.append(lambda g=g: emit_qk1(g, 0))
                        bg.append(lambda g=g: emit_qk1(g, 1))
                        for aa in range(4 * g, 4 * g + 4):
                            bg.append(lambda aa=aa: emit_v1(aa))
                    for m in range(TQ // 128):
                        bg.append(lambda jp=jp, m=m: emit_proj_m(jp, m))
                if bg:
                    bg.popleft()()

            exp_ctr = 0
            for j, pair, a in items:
                if a == 0:
                    pys[(j, pair)] = ps_y.tile(
                        [128, 4, 2, 64], dt.float32, tag="py",
                        name=f"py_{j}_{pair}"
                    )
                r = a - 4 * j
                off = 128 * r if r > 0 else 0
                p_s = ps_s.tile([128, 1024], dt.float32, tag="s")
                for ih, h in enumerate((2 * pair, 2 * pair + 1)):
                    nc.tensor.matmul(
                        p_s[:, TQ * ih + off:TQ * (ih + 1)],
                        qkT[32 * h:32 * (h + 1), 1, 128 * a:128 * (a + 1)],
                        qkT[32 * h:32 * (h + 1), 0, TQ * j + off:TQ * (j + 1)],
                        start=True, stop=True,
                        tile_position=(32 * h, 0),
                    )
                e_t = esb.tile([128, 1024], dt.bfloat16, tag="e")
                e_view = e_t.rearrange("p (i f) -> p i f", i=2)[:, :, off:]
                s_view = p_s.rearrange("p (i f) -> p i f", i=2)[:, :, off:]
                use_dve = (exp_ctr * DVE_EXP_NUM) % DVE_EXP_DEN < DVE_EXP_NUM
                exp_ctr += 1
                if use_dve:
                    nc.vector.tensor_scalar(
                        out=e_view.bitcast(dt.int16), in0=s_view,
                        scalar1=A_SCH, scalar2=B_SCH,
                        op0=mybir.AluOpType.mult, op1=mybir.AluOpType.add,
                    )
                else:
                    nc.scalar.activation(e_view, s_view, AF.Exp,
                                         scale=float(SCALE))
                if r >= 0:
                    # diag tile: zero the causal triangle (tk > tq) of E
                    sel = e_t.rearrange("p (i f) -> p i f", i=2)[:, :, off:off + 128]
                    nc.gpsimd.affine_select(
                        out=sel,
                        in_=sel,
                        compare_op=mybir.AluOpType.is_ge,
                        fill=0.0,
                        base=0,
                        pattern=[[0, 2], [1, 128]],
                        channel_multiplier=-1,
                    )
                pends.append((j, pair, e_t, a, r))
                if len(pends) > 6:
                    flush(pends.pop(0))
            for pp in pends:
                flush(pp)
            while bg:
                bg.popleft()()

    nc.compile()
    return nc


def _get_nc():
    if "nc" not in _cache:
        _cache["nc"] = _build()
    return _cache["nc"]


def run(inputs, trace=False):
    from concourse.bass_utils import run_bass_kernel_spmd

    nc = _get_nc()
    x = np.asarray(inputs["x"], dtype=np.float32)
    w_qkv = np.ascontiguousarray(np.asarray(inputs["w_qkv"], dtype=np.float32))
    b_qkv = np.ascontiguousarray(np.asarray(inputs["b_qkv"], dtype=np.float32))
    w_proj = np.ascontiguousarray(np.asarray(inputs["w_proj"], dtype=np.float32))
    b_proj = np.ascontiguousarray(np.asarray(inputs["b_proj"], dtype=np.float32))
    in_maps = [
        {
            "x": np.ascontiguousarray(x[b]),
            "w_qkv": w_qkv,
            "b_qkv": b_qkv,
            "w_proj": w_proj,
            "b_proj": b_proj,
        }
        for b in range(N_CORES)
    ]
    res = run_bass_kernel_spmd(
        nc, in_maps, core_ids=list(range(N_CORES)), trace=trace
    )
    out = np.stack([res.results[b]["y"] for b in range(N_CORES)], axis=0)
    return out, res


def kernel(**inputs) -> np.ndarray:
    out, _ = run(inputs, trace=False)
    return out


# revision 14
# speedup vs baseline: 1.0001x; 1.0001x over previous
"""Causal self-attention (B=8, T=2048, C=128, H=4, D=32) on 8 trn2 NeuronCores.

Sharding: data-parallel over batch - core b handles batch element b.

Per-core algorithm (all PE matmuls bf16; transposes fp32):
  xT = transpose(x) in bf16            # [C, T]
  qT, kT = (x @ Wq|k + b)^T bf16       # [C, {q,k}, T]
  v   = x @ Wv + bv                    # packed to vaug [128, NT, 4, 33]
                                       # (32 v dims + ones col = denominator)
  flat pipeline over (tq-block j, head pair, tk-tile a <= 4j+3):
      S^T[tk,tq] = kT_h.T @ qT_h       # K=32 row-packed, PSUM [128,1024]
      E = exp(S/sqrt(32))              # split: ACT exact exp | DVE 1-op
                                       # Schraudolph (int16 bitcast bf16)
      (diag: gpsimd affine_select zeroes the causal triangle of E)
      PV transposed, per (head, tq-tile m): psum_y[tq, h, 0:33] +=
          E-chunk[tk,tq-tile].T @ vaug[tk, h, 0:33]
  per j: denominators are psum_y[:, :, :, 32] (per-partition!) ->
  DVE reciprocal + broadcast-mult -> ynorm [tq, (h d)] fp32;
  PE transpose per tq-tile -> yT [(h d), tq] bf16; single proj matmul
  vs full w_proj bf16; bias add; DMA out. Proj work runs one block
  behind the attention pipeline as deferred background work.
"""

import sys

sys.path.insert(0, "/opt/trn_rl_repo")

import numpy as np

B, T, C = 8, 2048, 128
H, D = 4, 32
N_CORES = 8
TQ = 512          # tq block
NT = T // 128     # 16 tk tiles
NJ = T // TQ      # 4 tq blocks
SCALE = 1.0 / np.sqrt(D)
# Schraudolph exp: bf16(int16(s*A + B)); fold 1/sqrt(D) into A
A_SCH = float(SCALE * 128.0 / np.log(2.0))
B_SCH = float(127.0 * 128.0 - 7.42)
DVE_EXP_NUM = 5       # route 5/20 of exp items to DVE schraudolph
DVE_EXP_DEN = 20

_cache = {}


def _build():
    import concourse.bass as bass
    import concourse.mybir as mybir
    import concourse.tile as tile
    from concourse import bacc
    from concourse.masks import make_identity

    dt = mybir.dt
    AF = mybir.ActivationFunctionType
    nc = bacc.Bacc()

    x = nc.dram_tensor("x", [T, C], dt.float32, kind="ExternalInput")
    w_qkv = nc.dram_tensor("w_qkv", [C, 3 * C], dt.float32, kind="ExternalInput")
    b_qkv = nc.dram_tensor("b_qkv", [3 * C], dt.float32, kind="ExternalInput")
    w_proj = nc.dram_tensor("w_proj", [C, C], dt.float32, kind="ExternalInput")
    b_proj = nc.dram_tensor("b_proj", [C], dt.float32, kind="ExternalInput")
    y = nc.dram_tensor("y", [T, C], dt.float32, kind="ExternalOutput")

    with tile.TileContext(nc) as tc:
        with (
            nc.allow_low_precision(reason="bf16 matmuls + partial schraudolph exp; validated vs ref"),
            tc.tile_pool(name="const", bufs=1) as const,
            tc.tile_pool(name="big", bufs=1) as big,
            tc.tile_pool(name="sb", bufs=4) as sb,
            tc.tile_pool(name="esb", bufs=10) as esb,
            tc.tile_pool(name="ysb", bufs=3) as ysb,
            tc.tile_pool(name="ps_misc", bufs=2, space="PSUM") as ps_misc,
            tc.tile_pool(name="ps_s", bufs=2, space="PSUM") as ps_s,
            tc.tile_pool(name="ps_y", bufs=2, space="PSUM") as ps_y,
        ):
            # ---------------- critical-path constants ----------------
            ident = const.tile([128, 128], dt.float32)
            make_identity(nc, ident)

            # dummy exp so the ACT table set loads early
            dumm = const.tile([1, 1], dt.float32)
            nc.scalar.activation(dumm, ident[0:1, 0:1], AF.Exp)

            # persistent activations
            xT = big.tile([128, T], dt.bfloat16)        # [c, t]
            qkT = big.tile([128, 2, T], dt.bfloat16)    # [c, {q,k}, t]
            # vaug per tk-tile a, head h: [v_h (32) | 1.0] -> 33 wide
            vaug = big.tile([128, NT, 4, 33], dt.bfloat16)

            x_ts = {}

            def emit_xdma(a):
                x_t = sb.tile([128, 128], dt.float32, tag="xin",
                              name=f"x_t_{a}")
                nc.sync.dma_start(out=x_t, in_=x[128 * a:128 * (a + 1), :])
                x_ts[a] = x_t

            def emit_xtr(a):
                p_tr = ps_misc.tile([128, 128], dt.float32, tag="misc")
                nc.tensor.transpose(p_tr, x_ts.pop(a), ident)
                nc.vector.tensor_copy(xT[:, 128 * a:128 * (a + 1)], p_tr)

            def emit_x1(a):
                emit_xdma(a)
                emit_xtr(a)

            def emit_qk1(g, ch):
                p_qk = ps_misc.tile([128, TQ], dt.float32, tag="misc")
                nc.tensor.matmul(
                    p_qk,
                    w_r[:, 128 * ch:128 * (ch + 1)],
                    xT[:, TQ * g:TQ * (g + 1)],
                    start=True, stop=True,
                )
                nc.vector.tensor_scalar_add(
                    qkT[:, ch, TQ * g:TQ * (g + 1)], p_qk, bqk[:, ch:ch + 1]
                )

            def emit_v1(a):
                p_v = ps_misc.tile([128, 128], dt.float32, tag="misc")
                nc.tensor.matmul(
                    p_v,
                    xT[:, 128 * a:128 * (a + 1)],
                    w_v[:, :],
                    start=True, stop=True,
                )
                nc.vector.tensor_add(
                    vaug[:, a, :, 0:32],
                    p_v.rearrange("p (h d) -> p h d", h=4),
                    bvb.rearrange("p (h d) -> p h d", h=4),
                )

            def emit_xqk(g):
                for a in range(4 * g, 4 * g + 4):
                    emit_x1(a)
                emit_qk1(g, 0)
                emit_qk1(g, 1)

            def emit_v(g):
                for a in range(4 * g, 4 * g + 4):
                    emit_v1(a)

            # group-0 x tiles first in the DMA queue; weights right after
            for _a in range(4):
                emit_x1(_a)

            # biases: b_q/b_k as [128,1] per-partition columns
            bqk = const.tile([128, 2], dt.float32)
            nc.sync.dma_start(
                out=bqk, in_=b_qkv[0:256].rearrange("(j p) -> p j", p=128)
            )
            # weights in bf16
            w_sb = const.tile([128, 3 * C], dt.float32)
            nc.sync.dma_start(out=w_sb, in_=w_qkv[:, :])
            w_r = const.tile([128, 2 * C], dt.bfloat16)
            nc.vector.tensor_copy(w_r, w_sb[:, 0:256])
            w_v = const.tile([128, C], dt.bfloat16)
            nc.vector.tensor_copy(w_v, w_sb[:, 256:384])

            emit_qk1(0, 0)
            emit_qk1(0, 1)

            # ---------------- remaining constants ----------------
            wp_sb = const.tile([128, C], dt.float32)
            nc.sync.dma_start(out=wp_sb, in_=w_proj[:, :])
            wp_bf = const.tile([128, C], dt.bfloat16)
            nc.vector.tensor_copy(wp_bf, wp_sb)

            # broadcast tiles for free-dim biases (b_v, b_proj)
            brow = const.tile([1, 256], dt.float32)
            nc.sync.dma_start(out=brow[:, 0:128], in_=b_qkv[256:384][None, :])
            nc.sync.dma_start(out=brow[:, 128:256], in_=b_proj[:][None, :])
            brow_r = const.tile([1, 256], dt.float32r)
            nc.vector.tensor_copy(brow_r, brow)
            ones1_f = const.tile([1, 128], dt.float32)
            nc.vector.memset(ones1_f, 1.0)
            ones1 = const.tile([1, 128], dt.float32r)
            nc.vector.tensor_copy(ones1, ones1_f)
            p_b = ps_misc.tile([128, 256], dt.float32, tag="misc")
            nc.tensor.matmul(p_b, ones1, brow_r, start=True, stop=True)
            bvb = const.tile([128, 128], dt.float32)    # b_v broadcast
            bpb = const.tile([128, 128], dt.float32)    # b_proj broadcast
            nc.vector.tensor_copy(bvb, p_b[:, 0:128])
            nc.vector.tensor_copy(bpb, p_b[:, 128:256])

            nc.gpsimd.memset(vaug, 0.0)
            nc.gpsimd.memset(vaug[:, :, :, 32:33], 1.0)

            emit_v(0)
            emit_xqk(1)
            emit_v(1)

            # ---------------- attention pipeline ----------------
            pys = {}

            def emit_pvt(pend):
                # PV transposed: per (head, tq-tile m): stationary E chunk
                # [tk, tq-128], moving vaug [tk, 33] -> psum [tq, 33].
                # The py tile owns a full 2KB zero region: exactly one
                # start (first matmul) and one stop (last matmul) per tile;
                # in between, untouched bytes stay pending-zero so each
                # (m, ih) slice's first accumulate lands on zero.
                jp, pairp, e_p, a_p, r_p = pend
                py = pys[(jp, pairp)]
                first = (a_p == 0)
                last = (a_p == 4 * jp + 3)
                for ih in range(2):
                    h = 2 * pairp + ih
                    for m in range(max(r_p, 0), 4):
                        nc.tensor.matmul(
                            py[:, m, ih, 0:33],
                            e_p.rearrange("p (i f) -> p i f", i=2)[
                                :, ih, 128 * m:128 * (m + 1)],
                            vaug[:, a_p, h, :],
                            start=(first and ih == 0 and m == max(r_p, 0)),
                            stop=(last and ih == 1 and m == 3),
                            skip_group_check=True,
                        )

            def emit_norm_j(j):
                # denominators sit at free idx 32, per partition (= tq)
                ynb = ysb.tile([128, 4, 4, 32], dt.float32, tag="ynb",
                               name=f"ynb_{j}", bufs=2)
                for pair in range(2):
                    py = pys[(j, pair)]
                    r8 = ysb.tile([128, 4, 2, 1], dt.float32, tag="r8",
                                  name=f"r8_{j}_{pair}")
                    nc.vector.reciprocal(r8, py[:, :, :, 32:33])
                    rbc = bass.AP(
                        tensor=r8.tensor, offset=r8.offset,
                        ap=[r8.ap[0], [2, 4], [1, 2], [0, 32]],
                    )
                    nc.vector.tensor_tensor(
                        out=ynb[:, :, 2 * pair:2 * pair + 2, :],
                        in0=py[:, :, :, 0:32], in1=rbc,
                        op=mybir.AluOpType.mult,
                    )
                return ynb

            def emit_proj_m(j, m):
                ynb = ynbs[j]
                p_yt = ps_misc.tile([128, 128], dt.float32, tag="misc")
                nc.tensor.transpose(
                    p_yt, ynb[:, m, :, :].rearrange("p h d -> p (h d)"), ident
                )
                yt_sb = ysb.tile([128, 128], dt.bfloat16, tag="yt")
                nc.vector.tensor_copy(yt_sb, p_yt)
                p_o = ps_misc.tile([128, 128], dt.float32, tag="misc")
                nc.tensor.matmul(p_o, yt_sb, wp_bf, start=True, stop=True)
                o_t = sb.tile([128, 128], dt.float32, tag="out")
                nc.vector.tensor_add(o_t, p_o, bpb)
                t0 = TQ * j + 128 * m
                nc.sync.dma_start(out=y[t0:t0 + 128, :], in_=o_t)

            items = [
                (j, pair, a)
                for j in range(NJ)
                for pair in range(2)
                for a in range(4 * j + 4)
            ]
            pends = []
            ynbs = {}

            from collections import deque
            bg = deque()

            def flush(pend):
                emit_pvt(pend)
                jp, pairp, _, a_p, _ = pend
                if a_p == 4 * jp + 3 and pairp == 1:
                    ynbs[jp] = emit_norm_j(jp)
                    if jp + 2 < NJ:
                        g = jp + 2
                        for aa in range(4 * g, 4 * g + 4):
                            bg.append(lambda aa=aa: emit_xdma(aa))
                        for aa in range(4 * g, 4 * g + 4):
                            bg.append(lambda aa=aa: emit_xtr(aa))
                        bg.append(lambda g=g: emit_qk1(g, 0))
                        bg.append(lambda g=g: emit_qk1(g, 1))
                        for aa in range(4 * g, 4 * g + 4):
                            bg.append(lambda aa=aa: emit_v1(aa))
                    for m in range(TQ // 128):
                        bg.append(lambda jp=jp, m=m: emit_proj_m(jp, m))
                if bg:
                    bg.popleft()()

            exp_ctr = 0
            for j, pair, a in items:
                if a == 0:
                    pys[(j, pair)] = ps_y.tile(
                        [128, 4, 2, 64], dt.float32, tag="py",
                        name=f"py_{j}_{pair}"
                    )
                r = a - 4 * j
                off = 128 * r if r > 0 else 0
                p_s = ps_s.tile([128, 1024], dt.float32, tag="s")
                for ih, h in enumerate((2 * pair, 2 * pair + 1)):
                    nc.tensor.matmul(
                        p_s[:, TQ * ih + off:TQ * (ih + 1)],
                        qkT[32 * h:32 * (h + 1), 1, 128 * a:128 * (a + 1)],
                        qkT[32 * h:32 * (h + 1), 0, TQ * j + off:TQ * (j + 1)],
                        start=True, stop=True,
                        tile_position=(32 * h, 0),
                    )
                e_t = esb.tile([128, 1024], dt.bfloat16, tag="e")
                e_view = e_t.rearrange("p (i f) -> p i f", i=2)[:, :, off:]
                s_view = p_s.rearrange("p (i f) -> p i f", i=2)[:, :, off:]
                use_dve = (exp_ctr * DVE_EXP_NUM) % DVE_EXP_DEN < DVE_EXP_NUM
                exp_ctr += 1
                if use_dve:
                    nc.vector.tensor_scalar(
                        out=e_view.bitcast(dt.int16), in0=s_view,
                        scalar1=A_SCH, scalar2=B_SCH,
                        op0=mybir.AluOpType.mult, op1=mybir.AluOpType.add,
                    )
                else:
                    nc.scalar.activation(e_view, s_view, AF.Exp,
                                         scale=float(SCALE))
                if r >= 0:
                    # diag tile: zero the causal triangle (tk > tq) of E
                    sel = e_t.rearrange("p (i f) -> p i f", i=2)[:, :, off:off + 128]
                    nc.gpsimd.affine_select(
                        out=sel,
                        in_=sel,
                        compare_op=mybir.AluOpType.is_ge,
                        fill=0.0,
                        base=0,
                        pattern=[[0, 2], [1, 128]],
                        channel_multiplier=-1,
                    )
                pends.append((j, pair, e_t, a, r))
                if len(pends) > 7:
                    flush(pends.pop(0))
            for pp in pends:
                flush(pp)
            while bg:
                bg.popleft()()

    nc.compile()
    return nc


def _get_nc():
    if "nc" not in _cache:
        _cache["nc"] = _build()
    return _cache["nc"]


def run(inputs, trace=False):
    from concourse.bass_utils import run_bass_kernel_spmd

    nc = _get_nc()
    x = np.asarray(inputs["x"], dtype=np.float32)
    w_qkv = np.ascontiguousarray(np.asarray(inputs["w_qkv"], dtype=np.float32))
    b_qkv = np.ascontiguousarray(np.asarray(inputs["b_qkv"], dtype=np.float32))
    w_proj = np.ascontiguousarray(np.asarray(inputs["w_proj"], dtype=np.float32))
    b_proj = np.ascontiguousarray(np.asarray(inputs["b_proj"], dtype=np.float32))
    in_maps = [
        {
            "x": np.ascontiguousarray(x[b]),
            "w_qkv": w_qkv,
            "b_qkv": b_qkv,
            "w_proj": w_proj,
            "b_proj": b_proj,
        }
        for b in range(N_CORES)
    ]
    res = run_bass_kernel_spmd(
        nc, in_maps, core_ids=list(range(N_CORES)), trace=trace
    )
    out = np.stack([res.results[b]["y"] for b in range(N_CORES)], axis=0)
    return out, res


def kernel(**inputs) -> np.ndarray:
    out, _ = run(inputs, trace=False)
    return out
